# revision 32
# baseline (speedup 1.0000x reference)
"""Trainium2 Bass kernel for nn_Attention (B=2, N=2048, C=1024, H=16, D=64).

Sharding: tensor-parallel over heads — 16 heads / 8 cores = 2 heads per core.
Each core computes q/k/v projections for its 2 heads, attention, and its
partial contribution to the output projection (row-parallel w_out). The host
sums the 8 partials and adds b_out.

Layout: q/k stay transposed on-chip (feature dim on partitions; the host
supplies x pre-transposed in bf16). All matmul operands are bf16 (fp32r
streams at ~2 cycles/row from SBUF; bf16 runs the PE at 1 cycle/row) with
fp32 PSUM accumulation. V is produced directly in [token, feature] layout
by swapping the projection matmul operands (x chunk stationary, w_v moving)
— no PE transposes — and a ones column per head yields the softmax
denominator for free. Softmax skips max-subtraction (scores are O(1) by
construction).

Per m-tile, both heads' scores land in one [128,1024] PSUM tile so a single
ACTIVATE computes exp for both heads ((N+352)/1.2 ns cost model: fewer,
larger ACT ops). The softmax denominators are copied out of PSUM at once so
the PSUM accumulator recycles fast, then inverted with the fast custom-DVE
Newton-Raphson reciprocal (the iterative DVE reciprocal costs ~6 cyc/elem
on one lane). Output-projection results DMA to HBM straight from PSUM.

Scheduling: Tile freezes each engine's instruction order at schedule time,
so the emission is software-pipelined by hand:
 - attention chunk 0 of batch 0 is interleaved into the qkv projection
   itself — its 16 m-tile steps are windowed by k-chunk availability;
 - scores(mt+1) are emitted before attn@v(mt) so the PE covers exp latency;
 - the next batch's projection and the deferred output projection are
   drip-fed as "filler" PE work between attention steps.
DMA trigger ops ride the sync/gpsimd queues only so the ACT queue carries
exp almost exclusively (keeps the PE's HAM clock-gate warm: the attention
steady state is ACT-paced, and every ACT-queue bubble becomes a PE idle).
"""

import sys

for _p in ("/opt/trn_rl_repo", "/root/.axon_site/_ro/trn_rl_repo"):
    if _p not in sys.path:
        sys.path.append(_p)

import ml_dtypes
import numpy as np

import concourse.bass as bass
import concourse.tile as tile
from concourse import bacc, mybir
from concourse.bass_utils import run_bass_kernel_spmd

F32 = mybir.dt.float32
BF16 = mybir.dt.bfloat16
AFT = mybir.ActivationFunctionType
NP_BF16 = ml_dtypes.bfloat16

B, N, C = 2, 2048, 1024
H, D = 16, 64
NT = B * N
NCORES = 8
SCALE = D ** -0.5


def _build(loop_reps=1):
    nc = bacc.Bacc("TRN2", debug=False, target_bir_lowering=False, num_devices=NCORES)
    xT_d = nc.dram_tensor("xT", [C, NT], BF16, kind="ExternalInput").ap()
    wqkv_d = nc.dram_tensor("wqkvT", [C, 384], BF16, kind="ExternalInput").ap()
    bqkv_d = nc.dram_tensor("bqkv", [128, 3], F32, kind="ExternalInput").ap()
    bvrow_d = nc.dram_tensor("bvrow", [1, 128], F32, kind="ExternalInput").ap()
    wout_d = nc.dram_tensor("woutT", [128, C], BF16, kind="ExternalInput").ap()
    y_d = nc.dram_tensor("yT", [C, NT], BF16, kind="ExternalOutput").ap()

    with tile.TileContext(nc) as tc:
        with (
            tc.tile_pool(name="sb", bufs=1) as sp,
            tc.tile_pool(name="ps", bufs=1, space="PSUM") as ps,
        ):
            # ---- weights first: every projection matmul needs them ----
            wqkv_r = sp.tile([128, 3 * 8 * 128], BF16, tag="wqkv")
            for ct in range(8):
                eng = nc.sync if ct % 2 == 0 else nc.gpsimd
                eng.dma_start(
                    wqkv_r[:, ct * 384:(ct + 1) * 384],
                    wqkv_d[ct * 128:(ct + 1) * 128, :],
                )

            # ---- PE warm-up: ~4us of back-to-back dummy matmuls during the
            # initial DMA wait trips the HAM SHORT window, so the real
            # projection runs at 2.4 GHz instead of the cold 1.2/0.65 GHz.
            # The source is memset on DVE (its preamble finishes first) so
            # the warmup starts as early as possible.
            wu = sp.tile([64, 256], BF16, tag="wu")
            nc.vector.memset(wu[:], 0.0)
            wua = ps.tile([128, 512], F32, tag="acc", bufs=2, name="warm")
            for _ in range(20):
                nc.tensor.matmul(wua[0:64, 0:256], wu[:, 0:64], wu[:],
                                 start=True, stop=True)

            # dummy ACTIVATE so the ~2.7us exp table-set load also happens
            # during the DMA wait, not before the first real softmax
            ones_f = sp.tile([128, 1], F32, tag="onesf")
            nc.gpsimd.memset(ones_f[:], 1.0)
            wact = sp.tile([128, 1], F32, tag="wact")
            nc.scalar.activation(wact[:], ones_f[:], AFT.Exp)

            bias = sp.tile([128, 3], F32, tag="bias")
            nc.sync.dma_start(bias[:], bqkv_d[:, :])
            bvrow_dma = sp.tile([1, 128], F32, tag="bvrow")
            nc.sync.dma_start(bvrow_dma[:], bvrow_d[:, :])
            bvb = sp.tile([128, 128], F32, tag="bvb")
            nc.gpsimd.partition_broadcast(bvb[:], bvrow_dma[:])

            ones_r = sp.tile([128, 1], BF16, tag="ones")
            nc.vector.tensor_copy(ones_r[:], ones_f[:])

            wout_r = sp.tile([128, C], BF16, tag="wout")
            nc.scalar.dma_start(wout_r[:], wout_d[:, :])

            k_b = [
                sp.tile([128, N], BF16, tag="kv", bufs=2, name=f"k_{b}")
                for b in range(B)
            ]
            q_bc = [
                [sp.tile([128, 512], BF16, tag="qc", bufs=8, name=f"q{b}_{cch}")
                 for cch in range(4)]
                for b in range(B)
            ]
            # va[b][mt]: [128 tok, 130] = [h0 v (64) | ones | h1 v (64) | ones]
            # static tiles; the ones columns are prefilled once at startup
            # (DVE is idle then) instead of twice per tile mid-kernel.
            vaugs = {
                b: [sp.tile([128, 130], BF16, tag="vaug", bufs=32,
                            name=f"va{b}_{mt}")
                    for mt in range(16)]
                for b in range(B)
            }
            for b in range(B):
                for mt in range(16):
                    nc.vector.tensor_copy(vaugs[b][mt][:, 64:65], ones_r[:])
                    nc.vector.tensor_copy(vaugs[b][mt][:, 129:130], ones_r[:])
            fillers = []

            def drain(n=None):
                k = len(fillers) if n is None else min(n, len(fillers))
                for _ in range(k):
                    fillers.pop(0)()

            def emit_xt_dmas(b):
                # alternate between two DMA trigger queues so transfers of a
                # chunk overlap instead of serializing on one HWDGE queue.
                # bufs=64 holds both batches — trigger ops never block a
                # queue on a WAR wait for an old slot.
                xts = {}
                for ncq in range(4):
                    for ct in range(8):
                        t = sp.tile([128, 512], BF16, tag="xt", bufs=64,
                                    name=f"xt{b}_{ncq}_{ct}")
                        eng = nc.sync if ct % 2 == 0 else nc.gpsimd
                        eng.dma_start(
                            t[:],
                            xT_d[ct * 128:(ct + 1) * 128,
                                 b * N + ncq * 512:b * N + (ncq + 1) * 512],
                        )
                        xts[ncq, ct] = t
                return xts

            def qk_group_ops(b, xts, ncq, ot):
                """Closures: 8 accumulating matmuls + bias add for one
                512-chunk of the q/k/v row-block (feature-major)."""
                accs = {}

                def mk_mm(ct):
                    def go():
                        if ct == 0:
                            accs[0] = ps.tile([128, 512], F32, tag="acc", bufs=2,
                                              name=f"qacc{b}_{ncq}_{ot}")
                        nc.tensor.matmul(
                            accs[0][:],
                            wqkv_r[:, ct * 384 + ot * 128:ct * 384 + (ot + 1) * 128],
                            xts[ncq, ct][:],
                            start=(ct == 0),
                            stop=(ct == 7),
                        )
                    return go

                def fin():
                    if ot == 0:
                        dst = q_bc[b][ncq][:, :]
                    else:
                        dst = k_b[b][:, ncq * 512:(ncq + 1) * 512]
                    nc.vector.tensor_scalar_add(dst, accs[0][:], bias[:, ot:ot + 1])

                return [mk_mm(c) for c in range(8)] + [fin]

            def vproj_ops(b, xts, ncq):
                """Closures: token-major V projection for m-tiles
                4*ncq..4*ncq+3 (x chunk stationary, w_v moving), plus the
                augmented-V assembly (bias add; ones columns prefilled)."""
                ops = []
                for mt in range(4 * ncq, 4 * ncq + 4):
                    tt = mt % 4  # token tile within this 512 chunk
                    accs = {}

                    def mk_mm(ct, tt=tt, ncq=ncq, mt=mt, accs=accs):
                        def go():
                            if ct == 0:
                                accs[0] = ps.tile(
                                    [128, 512], F32, tag="acc", bufs=2,
                                    name=f"vacc{b}_{mt}")
                            nc.tensor.matmul(
                                accs[0][:, 0:128],
                                xts[ncq, ct][:, tt * 128:(tt + 1) * 128],
                                wqkv_r[:, ct * 384 + 256:ct * 384 + 384],
                                start=(ct == 0),
                                stop=(ct == 7),
                            )
                        return go

                    def fin(mt=mt, accs=accs):
                        va = vaugs[b][mt]
                        nc.vector.tensor_add(
                            va[:, 0:64], accs[0][:, 0:64], bvb[:, 0:64])
                        nc.vector.tensor_add(
                            va[:, 65:129], accs[0][:, 64:128], bvb[:, 64:128])

                    ops.extend([mk_mm(c) for c in range(8)] + [fin])
                return ops

            def queue_outproj(b, ncha, oc, inline=False):
                def mk(ct):
                    def go():
                        py = ps.tile([128, 512], F32, tag="acc", bufs=2,
                                     name=f"py{b}_{ncha}_{ct}")
                        nc.tensor.matmul(
                            py[:], wout_r[:, ct * 128:(ct + 1) * 128], oc[:],
                            start=True, stop=True,
                        )
                        yst = sp.tile([128, 512], BF16, tag="yst", bufs=4,
                                      name=f"yst{b}_{ncha}_{ct}")
                        # on the final (inline) chunk nothing else runs, so
                        # alternate copy engines to drain the outproj 2x as
                        # fast (the PSUM slot frees on the copy)
                        if inline and ct % 2:
                            nc.scalar.copy(yst[:], py[:])
                        else:
                            nc.vector.tensor_copy(yst[:], py[:])
                        deng = nc.sync if ct % 2 == 0 else nc.gpsimd
                        deng.dma_start(
                            y_d[ct * 128:(ct + 1) * 128,
                                b * N + ncha * 512:b * N + (ncha + 1) * 512],
                            yst[:],
                        )
                    return go

                for ct in range(8):
                    if inline:
                        mk(ct)()
                    else:
                        fillers.append(mk(ct))

            def att_chunk(b, ncha, fill, inline_out=False):
                """Generator: one yield per k-chunk window boundary.
                Caller must have emitted k-chunk w (and V m-tiles 4w..4w+3)
                before resuming window w."""
                kt = k_b[b]
                qt = q_bc[b][ncha]
                oc = sp.tile([128, 512], BF16, tag="ot", bufs=8,
                             name=f"oc{b}_{ncha}")
                po = [
                    ps.tile([65, 512], F32, tag="po", bufs=2,
                            name=f"po{b}_{ncha}_{h}")
                    for h in range(2)
                ]

                def scores(mt):
                    s = ps.tile([128, 1024], F32, tag="s", bufs=2,
                                name=f"s{b}_{ncha}_{mt}")
                    for h in range(2):
                        hs = slice(h * 64, (h + 1) * 64)
                        nc.tensor.matmul(
                            s[:, h * 512:(h + 1) * 512],
                            kt[hs, mt * 128:(mt + 1) * 128], qt[hs, :],
                            start=True, stop=True,
                        )
                    return s

                ss = scores(0)
                for mt in range(16):
                    p = sp.tile([128, 1024], BF16, tag="pt", bufs=3,
                                name=f"p{b}_{ncha}_{mt}")
                    nc.scalar.activation(p[:], ss[:], AFT.Exp, scale=SCALE)
                    if mt < 15:
                        if (mt + 1) % 4 == 0:
                            yield  # caller emits next k-chunk (+ V tiles)
                        ss = scores(mt + 1)
                    va = vaugs[b][mt]
                    for h in range(2):
                        nc.tensor.matmul(
                            po[h][:], va[:, 65 * h:65 * h + 65],
                            p[:, h * 512:(h + 1) * 512],
                            start=(mt == 0), stop=(mt == 15),
                        )
                    # defer the tail steps' fillers: their DVE ops would
                    # queue ahead of the normalize copies below and delay
                    # releasing the po accumulators for the next chunk
                    if mt < 14:
                        drain(fill)
                # batch-final chunks: the ACT queue idles right here (the
                # next batch's exps aren't ready yet), so borrow it for the
                # PSUM evacuation instead of lengthening the congested DVE
                # queue that also carries the next batch's projection fins.
                last_of_batch = ncha == 3
                for h in range(2):
                    hs = slice(h * 64, (h + 1) * 64)
                    # copy po out of PSUM at once so the accumulator bank
                    # recycles fast (next chunk's first attn@v reuses it)
                    pc = sp.tile([64, 512], F32, tag="pc", bufs=2,
                                 name=f"pc{b}_{ncha}_{h}")
                    dc = sp.tile([1, 512], F32, tag="dc", bufs=2,
                                 name=f"dc{b}_{ncha}_{h}")
                    if last_of_batch:
                        nc.scalar.copy(pc[:], po[h][0:64, :])
                        nc.scalar.copy(dc[:], po[h][64:65, :])
                    else:
                        nc.vector.tensor_copy(pc[:], po[h][0:64, :])
                        nc.vector.tensor_copy(dc[:], po[h][64:65, :])
                    rc = sp.tile([1, 512], F32, tag="rc", bufs=2,
                                 name=f"rc{b}_{ncha}_{h}")
                    nc.vector.reciprocal_approx_fast(rc[:], dc[:])
                    rb = sp.tile([64, 512], F32, tag="rb", bufs=2,
                                 name=f"rb{b}_{ncha}_{h}")
                    nc.gpsimd.partition_broadcast(rb[:], rc[:])
                    # the multiply reads/writes SBUF only, so it can run on
                    # the (idle) gpsimd queue; keep the final chunk's on DVE
                    # so its outproj isn't gated behind gpsimd y-DMA triggers
                    if inline_out:
                        nc.vector.tensor_mul(oc[hs, :], pc[:], rb[:])
                    else:
                        nc.gpsimd.tensor_mul(oc[hs, :], pc[:], rb[:])
                drain(2 * fill)  # the deferred mt=14/15 fillers
                queue_outproj(b, ncha, oc, inline=inline_out)

            # ---------- fused emission ----------
            def emit_body():
                xts0 = emit_xt_dmas(0)
                g = None
                for c in range(4):
                    for op in qk_group_ops(0, xts0, c, 1):  # k chunk c
                        op()
                    if c == 0:
                        for op in qk_group_ops(0, xts0, 0, 0):  # q chunk 0
                            op()
                        for op in vproj_ops(0, xts0, 0):  # v chunk 0
                            op()
                        g = att_chunk(0, 0, fill=0)
                        next(g)  # runs mt window 0 (mt 0-3), yields at boundary
                    else:
                        for op in qk_group_ops(0, xts0, c, 0):  # q chunk c
                            op()
                        for op in vproj_ops(0, xts0, c):  # v chunk c
                            op()
                        next(g, None)  # mt window c
                for _ in g:  # finish (normalize etc.)
                    pass

                # queue batch-1 projection as fillers, consumed inside
                # att(b0) chunks 1-3. All four k chunks go first: att(1,0)
                # needs the full k tile, so front-loading k minimizes how
                # deep into the filler list the b1 attention start reaches.
                xts1 = emit_xt_dmas(1)
                for c in range(4):
                    fillers.extend(qk_group_ops(1, xts1, c, 1))
                for c in range(4):
                    fillers.extend(qk_group_ops(1, xts1, c, 0))
                    fillers.extend(vproj_ops(1, xts1, c))

                att_chunks_left = 7  # 3 of batch 0 + 4 of batch 1
                for c in range(1, 4):
                    fill = min(4, max(1, -(-len(fillers) // (16 * att_chunks_left))))
                    for _ in att_chunk(0, c, fill=fill):
                        pass
                    att_chunks_left -= 1
                drain()  # flush any leftover b1 projection work
                for c in range(4):
                    last = c == 3
                    fill = min(4, max(1, -(-len(fillers) // (16 * att_chunks_left))))
                    for _ in att_chunk(1, c, fill=fill, inline_out=last):
                        pass
                    att_chunks_left -= 1
                drain()


            if loop_reps > 1:
                with tc.For_i(0, loop_reps, 1):
                    emit_body()
            else:
                emit_body()

    nc.compile()
    return nc


_NC_CACHE = None


def _get_nc():
    global _NC_CACHE
    if _NC_CACHE is None:
        _NC_CACHE = _build()
    return _NC_CACHE


def _in_maps(x, w_in, b_in, w_out):
    x_flat = np.asarray(x, dtype=np.float32).reshape(NT, C)
    xT = np.ascontiguousarray(x_flat.T).astype(NP_BF16)
    w_in = np.asarray(w_in, dtype=np.float32)
    b_in = np.asarray(b_in, dtype=np.float32)
    w_out = np.asarray(w_out, dtype=np.float32)
    maps = []
    for c in range(NCORES):
        h0, h1 = 2 * c, 2 * c + 1
        rows = np.r_[h0 * 64:(h0 + 1) * 64, h1 * 64:(h1 + 1) * 64]
        wq = w_in[rows, :]
        wk = w_in[C + rows, :]
        wv = w_in[2 * C + rows, :]
        wqkvT = np.ascontiguousarray(
            np.concatenate([wq, wk, wv], 0).T
        ).astype(NP_BF16)
        bqkv = np.ascontiguousarray(
            np.stack([b_in[rows], b_in[C + rows], b_in[2 * C + rows]], 1)
        )
        bvrow = np.ascontiguousarray(b_in[2 * C + rows][None, :])
        woutT = np.ascontiguousarray(w_out[:, rows].T).astype(NP_BF16)
        maps.append({"xT": xT, "wqkvT": wqkvT, "bqkv": bqkv,
                     "bvrow": bvrow, "woutT": woutT})
    return maps


def run_spmd(x, w_in, b_in, w_out, **kwargs):
    nc = _get_nc()
    maps = _in_maps(x, w_in, b_in, w_out)
    return run_bass_kernel_spmd(nc, maps, core_ids=list(range(NCORES)), **kwargs)


def kernel(x, w_in, b_in, w_out, b_out):
    res = run_spmd(x, w_in, b_in, w_out)
    yT = np.zeros((C, NT), dtype=np.float64)
    for r in res.results:
        yT += r["yT"].astype(np.float64)
    y = yT.T + np.asarray(b_out, dtype=np.float64)[None, :]
    return y.reshape(B, N, C).astype(np.float32)


# revision 33
# speedup vs baseline: 1.5409x; 1.5409x over previous
"""Trainium2 Bass kernel for nn_Attention (B=2, N=2048, C=1024, H=16, D=64).

Sharding: tensor-parallel over heads — 16 heads / 8 cores = 2 heads per core.
Each core computes q/k/v projections for its 2 heads, attention, and its
partial contribution to the output projection (row-parallel w_out). The host
sums the 8 partials and adds b_out.

Layout: q/k stay transposed on-chip (feature dim on partitions; the host
supplies x pre-transposed in bf16). All matmul operands are bf16 (fp32r
streams at ~2 cycles/row from SBUF; bf16 runs the PE at 1 cycle/row) with
fp32 PSUM accumulation. V is produced directly in [token, feature] layout
by swapping the projection matmul operands (x chunk stationary, w_v moving)
— no PE transposes — and a ones column per head yields the softmax
denominator for free. Softmax skips max-subtraction (scores are O(1) by
construction).

Per m-tile, both heads' scores land in one [128,1024] PSUM tile so a single
ACTIVATE computes exp for both heads ((N+352)/1.2 ns cost model: fewer,
larger ACT ops). The softmax denominators are copied out of PSUM at once so
the PSUM accumulator recycles fast, then inverted with the fast custom-DVE
Newton-Raphson reciprocal (the iterative DVE reciprocal costs ~6 cyc/elem
on one lane). Output-projection results DMA to HBM straight from PSUM.

Scheduling: Tile freezes each engine's instruction order at schedule time,
so the emission is software-pipelined by hand:
 - attention chunk 0 of batch 0 is interleaved into the qkv projection
   itself — its 16 m-tile steps are windowed by k-chunk availability;
 - scores(mt+1) are emitted before attn@v(mt) so the PE covers exp latency;
 - the next batch's projection and the deferred output projection are
   drip-fed as "filler" PE work between attention steps.
DMA trigger ops ride the sync/gpsimd queues only so the ACT queue carries
exp almost exclusively (keeps the PE's HAM clock-gate warm: the attention
steady state is ACT-paced, and every ACT-queue bubble becomes a PE idle).
"""

import sys

for _p in ("/opt/trn_rl_repo", "/root/.axon_site/_ro/trn_rl_repo"):
    if _p not in sys.path:
        sys.path.append(_p)

import ml_dtypes
import numpy as np

import concourse.bass as bass
import concourse.tile as tile
from concourse import bacc, mybir
from concourse.bass_utils import run_bass_kernel_spmd

F32 = mybir.dt.float32
BF16 = mybir.dt.bfloat16
AFT = mybir.ActivationFunctionType
NP_BF16 = ml_dtypes.bfloat16

B, N, C = 2, 2048, 1024
H, D = 16, 64
NT = B * N
NCORES = 8
SCALE = D ** -0.5


def _build(loop_reps=1):
    nc = bacc.Bacc("TRN2", debug=False, target_bir_lowering=False, num_devices=NCORES)
    xT_d = nc.dram_tensor("xT", [C, NT], BF16, kind="ExternalInput").ap()
    wqkv_d = nc.dram_tensor("wqkvT", [C, 384], BF16, kind="ExternalInput").ap()
    bqkv_d = nc.dram_tensor("bqkv", [128, 3], F32, kind="ExternalInput").ap()
    bvrow_d = nc.dram_tensor("bvrow", [1, 128], F32, kind="ExternalInput").ap()
    wout_d = nc.dram_tensor("woutT", [128, C], BF16, kind="ExternalInput").ap()
    y_d = nc.dram_tensor("yT", [C, NT], BF16, kind="ExternalOutput").ap()

    with tile.TileContext(nc) as tc:
        with (
            tc.tile_pool(name="sb", bufs=1) as sp,
            tc.tile_pool(name="ps", bufs=1, space="PSUM") as ps,
        ):
            # ---- weights first: every projection matmul needs them ----
            wqkv_r = sp.tile([128, 3 * 8 * 128], BF16, tag="wqkv")
            for ct in range(8):
                eng = nc.sync if ct % 2 == 0 else nc.gpsimd
                eng.dma_start(
                    wqkv_r[:, ct * 384:(ct + 1) * 384],
                    wqkv_d[ct * 128:(ct + 1) * 128, :],
                )

            # ---- PE warm-up: ~4us of back-to-back dummy matmuls during the
            # initial DMA wait trips the HAM SHORT window, so the real
            # projection runs at 2.4 GHz instead of the cold 1.2/0.65 GHz.
            # The source is memset on DVE (its preamble finishes first) so
            # the warmup starts as early as possible.
            wu = sp.tile([64, 256], BF16, tag="wu")
            nc.vector.memset(wu[:], 0.0)
            wua = ps.tile([128, 512], F32, tag="acc", bufs=2, name="warm")
            for _ in range(20):
                nc.tensor.matmul(wua[0:64, 0:256], wu[:, 0:64], wu[:],
                                 start=True, stop=True)

            # dummy ACTIVATE so the ~2.7us exp table-set load also happens
            # during the DMA wait, not before the first real softmax
            ones_f = sp.tile([128, 1], F32, tag="onesf")
            nc.gpsimd.memset(ones_f[:], 1.0)
            wact = sp.tile([128, 1], F32, tag="wact")
            nc.scalar.activation(wact[:], ones_f[:], AFT.Exp)

            bias = sp.tile([128, 3], F32, tag="bias")
            nc.sync.dma_start(bias[:], bqkv_d[:, :])
            bvrow_dma = sp.tile([1, 128], F32, tag="bvrow")
            nc.sync.dma_start(bvrow_dma[:], bvrow_d[:, :])
            bvb = sp.tile([128, 128], F32, tag="bvb")
            nc.gpsimd.partition_broadcast(bvb[:], bvrow_dma[:])

            ones_r = sp.tile([128, 1], BF16, tag="ones")
            nc.vector.tensor_copy(ones_r[:], ones_f[:])

            wout_r = sp.tile([128, C], BF16, tag="wout")
            nc.scalar.dma_start(wout_r[:], wout_d[:, :])

            k_b = [
                sp.tile([128, N], BF16, tag="kv", bufs=2, name=f"k_{b}")
                for b in range(B)
            ]
            q_bc = [
                [sp.tile([128, 512], BF16, tag="qc", bufs=8, name=f"q{b}_{cch}")
                 for cch in range(4)]
                for b in range(B)
            ]
            # va[b][mt]: [128 tok, 130] = [h0 v (64) | ones | h1 v (64) | ones]
            # static tiles; the ones columns are prefilled once at startup
            # (DVE is idle then) instead of twice per tile mid-kernel.
            vaugs = {
                b: [sp.tile([128, 130], BF16, tag="vaug", bufs=32,
                            name=f"va{b}_{mt}")
                    for mt in range(16)]
                for b in range(B)
            }
            for b in range(B):
                for mt in range(16):
                    nc.vector.tensor_copy(vaugs[b][mt][:, 64:65], ones_r[:])
                    nc.vector.tensor_copy(vaugs[b][mt][:, 129:130], ones_r[:])
            fillers = []

            def drain(n=None):
                k = len(fillers) if n is None else min(n, len(fillers))
                for _ in range(k):
                    fillers.pop(0)()

            def emit_xt_dmas(b):
                # alternate between two DMA trigger queues so transfers of a
                # chunk overlap instead of serializing on one HWDGE queue.
                # bufs=64 holds both batches — trigger ops never block a
                # queue on a WAR wait for an old slot.
                xts = {}
                for ncq in range(4):
                    for ct in range(8):
                        t = sp.tile([128, 512], BF16, tag="xt", bufs=64,
                                    name=f"xt{b}_{ncq}_{ct}")
                        eng = nc.sync if ct % 2 == 0 else nc.gpsimd
                        eng.dma_start(
                            t[:],
                            xT_d[ct * 128:(ct + 1) * 128,
                                 b * N + ncq * 512:b * N + (ncq + 1) * 512],
                        )
                        xts[ncq, ct] = t
                return xts

            def qk_group_ops(b, xts, ncq, ot):
                """Closures: 8 accumulating matmuls + bias add for one
                512-chunk of the q/k/v row-block (feature-major)."""
                accs = {}

                def mk_mm(ct):
                    def go():
                        if ct == 0:
                            accs[0] = ps.tile([128, 512], F32, tag="acc", bufs=2,
                                              name=f"qacc{b}_{ncq}_{ot}")
                        nc.tensor.matmul(
                            accs[0][:],
                            wqkv_r[:, ct * 384 + ot * 128:ct * 384 + (ot + 1) * 128],
                            xts[ncq, ct][:],
                            start=(ct == 0),
                            stop=(ct == 7),
                        )
                    return go

                def fin():
                    if ot == 0:
                        dst = q_bc[b][ncq][:, :]
                    else:
                        dst = k_b[b][:, ncq * 512:(ncq + 1) * 512]
                    nc.vector.tensor_scalar_add(dst, accs[0][:], bias[:, ot:ot + 1])

                return [mk_mm(c) for c in range(8)] + [fin]

            def vproj_ops(b, xts, ncq):
                """Closures: token-major V projection for m-tiles
                4*ncq..4*ncq+3 (x chunk stationary, w_v moving), plus the
                augmented-V assembly (bias add; ones columns prefilled)."""
                ops = []
                for mt in range(4 * ncq, 4 * ncq + 4):
                    tt = mt % 4  # token tile within this 512 chunk
                    accs = {}

                    def mk_mm(ct, tt=tt, ncq=ncq, mt=mt, accs=accs):
                        def go():
                            if ct == 0:
                                accs[0] = ps.tile(
                                    [128, 512], F32, tag="acc", bufs=2,
                                    name=f"vacc{b}_{mt}")
                            nc.tensor.matmul(
                                accs[0][:, 0:128],
                                xts[ncq, ct][:, tt * 128:(tt + 1) * 128],
                                wqkv_r[:, ct * 384 + 256:ct * 384 + 384],
                                start=(ct == 0),
                                stop=(ct == 7),
                            )
                        return go

                    def fin(mt=mt, accs=accs):
                        va = vaugs[b][mt]
                        nc.vector.tensor_add(
                            va[:, 0:64], accs[0][:, 0:64], bvb[:, 0:64])
                        nc.vector.tensor_add(
                            va[:, 65:129], accs[0][:, 64:128], bvb[:, 64:128])

                    ops.extend([mk_mm(c) for c in range(8)] + [fin])
                return ops

            def queue_outproj(b, ncha, oc, inline=False):
                def mk(ct):
                    def go():
                        py = ps.tile([128, 512], F32, tag="acc", bufs=2,
                                     name=f"py{b}_{ncha}_{ct}")
                        nc.tensor.matmul(
                            py[:], wout_r[:, ct * 128:(ct + 1) * 128], oc[:],
                            start=True, stop=True,
                        )
                        yst = sp.tile([128, 512], BF16, tag="yst", bufs=4,
                                      name=f"yst{b}_{ncha}_{ct}")
                        # on the final (inline) chunk nothing else runs, so
                        # alternate copy engines to drain the outproj 2x as
                        # fast (the PSUM slot frees on the copy)
                        if inline and ct % 2:
                            nc.scalar.copy(yst[:], py[:])
                        else:
                            nc.vector.tensor_copy(yst[:], py[:])
                        deng = nc.sync if ct % 2 == 0 else nc.gpsimd
                        deng.dma_start(
                            y_d[ct * 128:(ct + 1) * 128,
                                b * N + ncha * 512:b * N + (ncha + 1) * 512],
                            yst[:],
                        )
                    return go

                for ct in range(8):
                    if inline:
                        mk(ct)()
                    else:
                        fillers.append(mk(ct))

            def att_chunk(b, ncha, fill, inline_out=False):
                """Generator: one yield per k-chunk window boundary.
                Caller must have emitted k-chunk w (and V m-tiles 4w..4w+3)
                before resuming window w."""
                kt = k_b[b]
                qt = q_bc[b][ncha]
                oc = sp.tile([128, 512], BF16, tag="ot", bufs=8,
                             name=f"oc{b}_{ncha}")
                po = [
                    ps.tile([65, 512], F32, tag="po", bufs=2,
                            name=f"po{b}_{ncha}_{h}")
                    for h in range(2)
                ]

                def scores(mt):
                    s = ps.tile([128, 1024], F32, tag="s", bufs=2,
                                name=f"s{b}_{ncha}_{mt}")
                    for h in range(2):
                        hs = slice(h * 64, (h + 1) * 64)
                        nc.tensor.matmul(
                            s[:, h * 512:(h + 1) * 512],
                            kt[hs, mt * 128:(mt + 1) * 128], qt[hs, :],
                            start=True, stop=True,
                        )
                    return s

                ss = scores(0)
                for mt in range(16):
                    p = sp.tile([128, 1024], BF16, tag="pt", bufs=3,
                                name=f"p{b}_{ncha}_{mt}")
                    nc.scalar.activation(p[:], ss[:], AFT.Exp, scale=SCALE)
                    if mt < 15:
                        if (mt + 1) % 4 == 0:
                            yield  # caller emits next k-chunk (+ V tiles)
                        ss = scores(mt + 1)
                    va = vaugs[b][mt]
                    for h in range(2):
                        nc.tensor.matmul(
                            po[h][:], va[:, 65 * h:65 * h + 65],
                            p[:, h * 512:(h + 1) * 512],
                            start=(mt == 0), stop=(mt == 15),
                        )
                    # defer the tail steps' fillers: their DVE ops would
                    # queue ahead of the normalize copies below and delay
                    # releasing the po accumulators for the next chunk
                    if mt < 14:
                        drain(fill)
                # batch-final chunks: the ACT queue idles right here (the
                # next batch's exps aren't ready yet), so borrow it for the
                # PSUM evacuation instead of lengthening the congested DVE
                # queue that also carries the next batch's projection fins.
                last_of_batch = ncha == 3
                for h in range(2):
                    hs = slice(h * 64, (h + 1) * 64)
                    # copy po out of PSUM at once so the accumulator bank
                    # recycles fast (next chunk's first attn@v reuses it)
                    pc = sp.tile([64, 512], F32, tag="pc", bufs=2,
                                 name=f"pc{b}_{ncha}_{h}")
                    dc = sp.tile([1, 512], F32, tag="dc", bufs=2,
                                 name=f"dc{b}_{ncha}_{h}")
                    if last_of_batch:
                        nc.scalar.copy(pc[:], po[h][0:64, :])
                        nc.scalar.copy(dc[:], po[h][64:65, :])
                    else:
                        nc.vector.tensor_copy(pc[:], po[h][0:64, :])
                        nc.vector.tensor_copy(dc[:], po[h][64:65, :])
                    rc = sp.tile([1, 512], F32, tag="rc", bufs=2,
                                 name=f"rc{b}_{ncha}_{h}")
                    nc.vector.reciprocal_approx_fast(rc[:], dc[:])
                    rb = sp.tile([64, 512], F32, tag="rb", bufs=2,
                                 name=f"rb{b}_{ncha}_{h}")
                    nc.gpsimd.partition_broadcast(rb[:], rc[:])
                    nc.vector.tensor_mul(oc[hs, :], pc[:], rb[:])
                drain(2 * fill)  # the deferred mt=14/15 fillers
                queue_outproj(b, ncha, oc, inline=inline_out)

            # ---------- fused emission ----------
            def emit_body():
                xts0 = emit_xt_dmas(0)
                g = None
                for c in range(4):
                    for op in qk_group_ops(0, xts0, c, 1):  # k chunk c
                        op()
                    if c == 0:
                        for op in qk_group_ops(0, xts0, 0, 0):  # q chunk 0
                            op()
                        for op in vproj_ops(0, xts0, 0):  # v chunk 0
                            op()
                        g = att_chunk(0, 0, fill=0)
                        next(g)  # runs mt window 0 (mt 0-3), yields at boundary
                    else:
                        for op in qk_group_ops(0, xts0, c, 0):  # q chunk c
                            op()
                        for op in vproj_ops(0, xts0, c):  # v chunk c
                            op()
                        next(g, None)  # mt window c
                for _ in g:  # finish (normalize etc.)
                    pass

                # queue batch-1 projection as fillers, consumed inside
                # att(b0) chunks 1-3. All four k chunks go first: att(1,0)
                # needs the full k tile, so front-loading k minimizes how
                # deep into the filler list the b1 attention start reaches.
                xts1 = emit_xt_dmas(1)
                for c in range(4):
                    fillers.extend(qk_group_ops(1, xts1, c, 1))
                for c in range(4):
                    fillers.extend(qk_group_ops(1, xts1, c, 0))
                    fillers.extend(vproj_ops(1, xts1, c))

                att_chunks_left = 7  # 3 of batch 0 + 4 of batch 1
                for c in range(1, 4):
                    fill = min(4, max(1, -(-len(fillers) // (16 * att_chunks_left))))
                    for _ in att_chunk(0, c, fill=fill):
                        pass
                    att_chunks_left -= 1
                drain()  # flush any leftover b1 projection work
                for c in range(4):
                    last = c == 3
                    fill = min(4, max(1, -(-len(fillers) // (16 * att_chunks_left))))
                    for _ in att_chunk(1, c, fill=fill, inline_out=last):
                        pass
                    att_chunks_left -= 1
                drain()


            if loop_reps > 1:
                with tc.For_i(0, loop_reps, 1):
                    emit_body()
            else:
                emit_body()

    nc.compile()
    return nc


_NC_CACHE = None


def _get_nc():
    global _NC_CACHE
    if _NC_CACHE is None:
        _NC_CACHE = _build()
    return _NC_CACHE


def _in_maps(x, w_in, b_in, w_out):
    x_flat = np.asarray(x, dtype=np.float32).reshape(NT, C)
    xT = np.ascontiguousarray(x_flat.T).astype(NP_BF16)
    w_in = np.asarray(w_in, dtype=np.float32)
    b_in = np.asarray(b_in, dtype=np.float32)
    w_out = np.asarray(w_out, dtype=np.float32)
    maps = []
    for c in range(NCORES):
        h0, h1 = 2 * c, 2 * c + 1
        rows = np.r_[h0 * 64:(h0 + 1) * 64, h1 * 64:(h1 + 1) * 64]
        wq = w_in[rows, :]
        wk = w_in[C + rows, :]
        wv = w_in[2 * C + rows, :]
        wqkvT = np.ascontiguousarray(
            np.concatenate([wq, wk, wv], 0).T
        ).astype(NP_BF16)
        bqkv = np.ascontiguousarray(
            np.stack([b_in[rows], b_in[C + rows], b_in[2 * C + rows]], 1)
        )
        bvrow = np.ascontiguousarray(b_in[2 * C + rows][None, :])
        woutT = np.ascontiguousarray(w_out[:, rows].T).astype(NP_BF16)
        maps.append({"xT": xT, "wqkvT": wqkvT, "bqkv": bqkv,
                     "bvrow": bvrow, "woutT": woutT})
    return maps


def run_spmd(x, w_in, b_in, w_out, **kwargs):
    nc = _get_nc()
    maps = _in_maps(x, w_in, b_in, w_out)
    return run_bass_kernel_spmd(nc, maps, core_ids=list(range(NCORES)), **kwargs)


def kernel(x, w_in, b_in, w_out, b_out):
    res = run_spmd(x, w_in, b_in, w_out)
    yT = np.zeros((C, NT), dtype=np.float64)
    for r in res.results:
        yT += r["yT"].astype(np.float64)
    y = yT.T + np.asarray(b_out, dtype=np.float64)[None, :]
    return y.reshape(B, N, C).astype(np.float32)


# revision 37
# speedup vs baseline: 1.6297x; 1.0577x over previous
"""Trainium2 Bass kernel for nn_Attention (B=2, N=2048, C=1024, H=16, D=64).

Sharding: tensor-parallel over heads — 16 heads / 8 cores = 2 heads per core.
Each core computes q/k/v projections for its 2 heads, attention, and its
partial contribution to the output projection (row-parallel w_out). The host
sums the 8 partials and adds b_out.

Layout: q/k stay transposed on-chip (feature dim on partitions; the host
supplies x pre-transposed in bf16). All matmul operands are bf16 (fp32r
streams at ~2 cycles/row from SBUF; bf16 runs the PE at 1 cycle/row) with
fp32 PSUM accumulation. V is produced directly in [token, feature] layout
by swapping the projection matmul operands (x chunk stationary, w_v moving)
— no PE transposes — and a ones column per head yields the softmax
denominator for free. Softmax skips max-subtraction (scores are O(1) by
construction).

Per m-tile, both heads' scores land in one [128,1024] PSUM tile so a single
ACTIVATE computes exp for both heads ((N+352)/1.2 ns cost model: fewer,
larger ACT ops). The softmax denominators are copied out of PSUM at once so
the PSUM accumulator recycles fast, then inverted with the fast custom-DVE
Newton-Raphson reciprocal (the iterative DVE reciprocal costs ~6 cyc/elem
on one lane). Output-projection results DMA to HBM straight from PSUM.

Scheduling: Tile freezes each engine's instruction order at schedule time,
so the emission is software-pipelined by hand:
 - attention chunk 0 of batch 0 is interleaved into the qkv projection
   itself — its 16 m-tile steps are windowed by k-chunk availability;
 - scores(mt+1) are emitted before attn@v(mt) so the PE covers exp latency;
 - the next batch's projection and the deferred output projection are
   drip-fed as "filler" PE work between attention steps.
DMA trigger ops ride the sync/gpsimd queues only so the ACT queue carries
exp almost exclusively (keeps the PE's HAM clock-gate warm: the attention
steady state is ACT-paced, and every ACT-queue bubble becomes a PE idle).
"""

import sys

for _p in ("/opt/trn_rl_repo", "/root/.axon_site/_ro/trn_rl_repo"):
    if _p not in sys.path:
        sys.path.append(_p)

import ml_dtypes
import numpy as np

import concourse.bass as bass
import concourse.tile as tile
from concourse import bacc, mybir
from concourse.bass_utils import run_bass_kernel_spmd

F32 = mybir.dt.float32
BF16 = mybir.dt.bfloat16
AFT = mybir.ActivationFunctionType
NP_BF16 = ml_dtypes.bfloat16

B, N, C = 2, 2048, 1024
H, D = 16, 64
NT = B * N
NCORES = 8
SCALE = D ** -0.5


def _build(loop_reps=1):
    nc = bacc.Bacc("TRN2", debug=False, target_bir_lowering=False, num_devices=NCORES)
    xT_d = nc.dram_tensor("xT", [C, NT], BF16, kind="ExternalInput").ap()
    wqkv_d = nc.dram_tensor("wqkvT", [C, 384], BF16, kind="ExternalInput").ap()
    bqkv_d = nc.dram_tensor("bqkv", [128, 3], F32, kind="ExternalInput").ap()
    bvrow_d = nc.dram_tensor("bvrow", [1, 128], F32, kind="ExternalInput").ap()
    wout_d = nc.dram_tensor("woutT", [128, C], BF16, kind="ExternalInput").ap()
    y_d = nc.dram_tensor("yT", [C, NT], BF16, kind="ExternalOutput").ap()

    with tile.TileContext(nc) as tc:
        with (
            tc.tile_pool(name="sb", bufs=1) as sp,
            tc.tile_pool(name="ps", bufs=1, space="PSUM") as ps,
        ):
            # ---- weights first: every projection matmul needs them ----
            wqkv_r = sp.tile([128, 3 * 8 * 128], BF16, tag="wqkv")
            for ct in range(8):
                eng = nc.sync if ct % 2 == 0 else nc.gpsimd
                eng.dma_start(
                    wqkv_r[:, ct * 384:(ct + 1) * 384],
                    wqkv_d[ct * 128:(ct + 1) * 128, :],
                )

            # ---- PE warm-up: ~4us of back-to-back dummy matmuls during the
            # initial DMA wait trips the HAM SHORT window, so the real
            # projection runs at 2.4 GHz instead of the cold 1.2/0.65 GHz.
            # The source is memset on DVE (its preamble finishes first) so
            # the warmup starts as early as possible.
            wu = sp.tile([64, 256], BF16, tag="wu")
            nc.vector.memset(wu[:], 0.0)
            wua = ps.tile([128, 512], F32, tag="acc", bufs=2, name="warm")
            for _ in range(20):
                nc.tensor.matmul(wua[0:64, 0:256], wu[:, 0:64], wu[:],
                                 start=True, stop=True)

            # dummy ACTIVATE so the ~2.7us exp table-set load also happens
            # during the DMA wait, not before the first real softmax
            ones_f = sp.tile([128, 1], F32, tag="onesf")
            nc.gpsimd.memset(ones_f[:], 1.0)
            wact = sp.tile([128, 1], F32, tag="wact")
            nc.scalar.activation(wact[:], ones_f[:], AFT.Exp)

            bias = sp.tile([128, 3], F32, tag="bias")
            nc.sync.dma_start(bias[:], bqkv_d[:, :])
            bvrow_dma = sp.tile([1, 128], F32, tag="bvrow")
            nc.sync.dma_start(bvrow_dma[:], bvrow_d[:, :])
            bvb = sp.tile([128, 128], F32, tag="bvb")
            nc.gpsimd.partition_broadcast(bvb[:], bvrow_dma[:])

            ones_r = sp.tile([128, 1], BF16, tag="ones")
            nc.vector.tensor_copy(ones_r[:], ones_f[:])

            wout_r = sp.tile([128, C], BF16, tag="wout")
            nc.scalar.dma_start(wout_r[:], wout_d[:, :])

            k_b = [
                sp.tile([128, N], BF16, tag="kv", bufs=2, name=f"k_{b}")
                for b in range(B)
            ]
            q_bc = [
                [sp.tile([128, 512], BF16, tag="qc", bufs=8, name=f"q{b}_{cch}")
                 for cch in range(4)]
                for b in range(B)
            ]
            # va[b][mt]: [128 tok, 130] = [h0 v (64) | ones | h1 v (64) | ones]
            # static tiles; the ones columns are prefilled once at startup
            # (DVE is idle then) instead of twice per tile mid-kernel.
            vaugs = {
                b: [sp.tile([128, 130], BF16, tag="vaug", bufs=32,
                            name=f"va{b}_{mt}")
                    for mt in range(16)]
                for b in range(B)
            }
            for b in range(B):
                for mt in range(16):
                    nc.vector.tensor_copy(vaugs[b][mt][:, 64:65], ones_r[:])
                    nc.vector.tensor_copy(vaugs[b][mt][:, 129:130], ones_r[:])
            fillers = []
            consumed = [0]

            def drain(n=None):
                k = len(fillers) if n is None else min(n, len(fillers))
                for _ in range(k):
                    fillers.pop(0)()
                consumed[0] += k

            def drain_to(target):
                # emit fillers until `target` total have been emitted —
                # guarantees everything queued up to that point is emitted
                # (Tile dependencies follow emission order)
                while consumed[0] < target and fillers:
                    fillers.pop(0)()
                    consumed[0] += 1

            def emit_xt_dmas(b):
                # alternate between two DMA trigger queues so transfers of a
                # chunk overlap instead of serializing on one HWDGE queue.
                # bufs=64 holds both batches — trigger ops never block a
                # queue on a WAR wait for an old slot.
                xts = {}
                for ncq in range(4):
                    for ct in range(8):
                        t = sp.tile([128, 512], BF16, tag="xt", bufs=64,
                                    name=f"xt{b}_{ncq}_{ct}")
                        eng = nc.sync if ct % 2 == 0 else nc.gpsimd
                        eng.dma_start(
                            t[:],
                            xT_d[ct * 128:(ct + 1) * 128,
                                 b * N + ncq * 512:b * N + (ncq + 1) * 512],
                        )
                        xts[ncq, ct] = t
                return xts

            def qk_group_ops(b, xts, ncq, ot):
                """Closures: 8 accumulating matmuls + bias add for one
                512-chunk of the q/k/v row-block (feature-major)."""
                accs = {}

                def mk_mm(ct):
                    def go():
                        if ct == 0:
                            accs[0] = ps.tile([128, 512], F32, tag="acc", bufs=2,
                                              name=f"qacc{b}_{ncq}_{ot}")
                        nc.tensor.matmul(
                            accs[0][:],
                            wqkv_r[:, ct * 384 + ot * 128:ct * 384 + (ot + 1) * 128],
                            xts[ncq, ct][:],
                            start=(ct == 0),
                            stop=(ct == 7),
                        )
                    return go

                def fin():
                    if ot == 0:
                        dst = q_bc[b][ncq][:, :]
                    else:
                        dst = k_b[b][:, ncq * 512:(ncq + 1) * 512]
                    nc.vector.tensor_scalar_add(dst, accs[0][:], bias[:, ot:ot + 1])

                return [mk_mm(c) for c in range(8)] + [fin]

            def vproj_ops(b, xts, ncq):
                """Closures: token-major V projection for m-tiles
                4*ncq..4*ncq+3 (x chunk stationary, w_v moving), plus the
                augmented-V assembly (bias add; ones columns prefilled)."""
                ops = []
                for mt in range(4 * ncq, 4 * ncq + 4):
                    tt = mt % 4  # token tile within this 512 chunk
                    accs = {}

                    def mk_mm(ct, tt=tt, ncq=ncq, mt=mt, accs=accs):
                        def go():
                            if ct == 0:
                                accs[0] = ps.tile(
                                    [128, 512], F32, tag="acc", bufs=2,
                                    name=f"vacc{b}_{mt}")
                            nc.tensor.matmul(
                                accs[0][:, 0:128],
                                xts[ncq, ct][:, tt * 128:(tt + 1) * 128],
                                wqkv_r[:, ct * 384 + 256:ct * 384 + 384],
                                start=(ct == 0),
                                stop=(ct == 7),
                            )
                        return go

                    def fin(mt=mt, accs=accs):
                        va = vaugs[b][mt]
                        nc.vector.tensor_add(
                            va[:, 0:64], accs[0][:, 0:64], bvb[:, 0:64])
                        nc.vector.tensor_add(
                            va[:, 65:129], accs[0][:, 64:128], bvb[:, 64:128])

                    ops.extend([mk_mm(c) for c in range(8)] + [fin])
                return ops

            def queue_outproj(b, ncha, oc, inline=False):
                def mk(ct):
                    def go():
                        py = ps.tile([128, 512], F32, tag="acc", bufs=2,
                                     name=f"py{b}_{ncha}_{ct}")
                        nc.tensor.matmul(
                            py[:], wout_r[:, ct * 128:(ct + 1) * 128], oc[:],
                            start=True, stop=True,
                        )
                        yst = sp.tile([128, 512], BF16, tag="yst", bufs=4,
                                      name=f"yst{b}_{ncha}_{ct}")
                        # on the final (inline) chunk nothing else runs, so
                        # alternate copy engines to drain the outproj 2x as
                        # fast (the PSUM slot frees on the copy)
                        if inline and ct % 2:
                            nc.scalar.copy(yst[:], py[:])
                        else:
                            nc.vector.tensor_copy(yst[:], py[:])
                        deng = nc.sync if ct % 2 == 0 else nc.gpsimd
                        deng.dma_start(
                            y_d[ct * 128:(ct + 1) * 128,
                                b * N + ncha * 512:b * N + (ncha + 1) * 512],
                            yst[:],
                        )
                    return go

                for ct in range(8):
                    if inline:
                        mk(ct)()
                    else:
                        fillers.append(mk(ct))

            def att_chunk(b, ncha, fill, inline_out=False):
                """Generator: one yield per k-chunk window boundary.
                Caller must have emitted k-chunk w (and V m-tiles 4w..4w+3)
                before resuming window w."""
                kt = k_b[b]
                qt = q_bc[b][ncha]
                oc = sp.tile([128, 512], BF16, tag="ot", bufs=8,
                             name=f"oc{b}_{ncha}")
                po = [
                    ps.tile([65, 512], F32, tag="po", bufs=2,
                            name=f"po{b}_{ncha}_{h}")
                    for h in range(2)
                ]

                def scores(mt):
                    s = ps.tile([128, 1024], F32, tag="s", bufs=2,
                                name=f"s{b}_{ncha}_{mt}")
                    for h in range(2):
                        hs = slice(h * 64, (h + 1) * 64)
                        nc.tensor.matmul(
                            s[:, h * 512:(h + 1) * 512],
                            kt[hs, mt * 128:(mt + 1) * 128], qt[hs, :],
                            start=True, stop=True,
                        )
                    return s

                ss = scores(0)
                for mt in range(16):
                    p = sp.tile([128, 1024], BF16, tag="pt", bufs=3,
                                name=f"p{b}_{ncha}_{mt}")
                    nc.scalar.activation(p[:], ss[:], AFT.Exp, scale=SCALE)
                    if mt < 15:
                        if (mt + 1) % 4 == 0:
                            yield  # caller emits next k-chunk (+ V tiles)
                        ss = scores(mt + 1)
                    va = vaugs[b][mt]
                    for h in range(2):
                        nc.tensor.matmul(
                            po[h][:], va[:, 65 * h:65 * h + 65],
                            p[:, h * 512:(h + 1) * 512],
                            start=(mt == 0), stop=(mt == 15),
                        )
                    # defer the tail steps' fillers: their DVE ops would
                    # queue ahead of the normalize copies below and delay
                    # releasing the po accumulators for the next chunk
                    if mt < 14:
                        drain(fill)
                # batch-final chunks: the ACT queue idles right here (the
                # next batch's exps aren't ready yet), so borrow it for the
                # PSUM evacuation instead of lengthening the congested DVE
                # queue that also carries the next batch's projection fins.
                last_of_batch = ncha == 3
                for h in range(2):
                    hs = slice(h * 64, (h + 1) * 64)
                    # copy po out of PSUM at once so the accumulator bank
                    # recycles fast (next chunk's first attn@v reuses it)
                    pc = sp.tile([64, 512], F32, tag="pc", bufs=2,
                                 name=f"pc{b}_{ncha}_{h}")
                    dc = sp.tile([1, 512], F32, tag="dc", bufs=2,
                                 name=f"dc{b}_{ncha}_{h}")
                    if last_of_batch:
                        nc.scalar.copy(pc[:], po[h][0:64, :])
                        nc.scalar.copy(dc[:], po[h][64:65, :])
                    else:
                        nc.vector.tensor_copy(pc[:], po[h][0:64, :])
                        nc.vector.tensor_copy(dc[:], po[h][64:65, :])
                    rc = sp.tile([1, 512], F32, tag="rc", bufs=2,
                                 name=f"rc{b}_{ncha}_{h}")
                    nc.vector.reciprocal_approx_fast(rc[:], dc[:])
                    rb = sp.tile([64, 512], F32, tag="rb", bufs=2,
                                 name=f"rb{b}_{ncha}_{h}")
                    nc.gpsimd.partition_broadcast(rb[:], rc[:])
                    nc.vector.tensor_mul(oc[hs, :], pc[:], rb[:])
                drain(2 * fill)  # the deferred mt=14/15 fillers
                queue_outproj(b, ncha, oc, inline=inline_out)

            # ---------- fused emission ----------
            def emit_body():
                xts0 = emit_xt_dmas(0)
                g = None
                for c in range(4):
                    for op in qk_group_ops(0, xts0, c, 1):  # k chunk c
                        op()
                    if c == 0:
                        for op in qk_group_ops(0, xts0, 0, 0):  # q chunk 0
                            op()
                        for op in vproj_ops(0, xts0, 0):  # v chunk 0
                            op()
                        g = att_chunk(0, 0, fill=0)
                        next(g)  # runs mt window 0 (mt 0-3), yields at boundary
                    else:
                        for op in qk_group_ops(0, xts0, c, 0):  # q chunk c
                            op()
                        for op in vproj_ops(0, xts0, c):  # v chunk c
                            op()
                        next(g, None)  # mt window c
                for _ in g:  # finish (normalize etc.)
                    pass

                # queue batch-1 projection as fillers, consumed inside
                # att(b0) chunks 1-3. All four k chunks go first: att(1,0)
                # needs the full k tile, so front-loading k minimizes how
                # deep into the filler list the b1 attention start reaches.
                # b1 filler order: all k chunks, then q0 and ALL v chunks
                # (any b1 attention chunk's m-tile loop spans every key, so
                # it reads all 16 va tiles), then the remaining q chunks.
                xts1 = emit_xt_dmas(1)
                for c in range(4):
                    fillers.extend(qk_group_ops(1, xts1, c, 1))
                fillers.extend(qk_group_ops(1, xts1, 0, 0))
                vpos = {}
                for w in range(4):
                    fillers.extend(vproj_ops(1, xts1, w))
                    vpos[w] = consumed[0] + len(fillers)
                qpos = {}
                for c in range(1, 4):
                    fillers.extend(qk_group_ops(1, xts1, c, 0))
                    qpos[c] = consumed[0] + len(fillers)

                att_chunks_left = 7  # 3 of batch 0 + 4 of batch 1
                for c in range(1, 4):
                    fill = min(4, max(1, -(-len(fillers) // (16 * att_chunks_left))))
                    for _ in att_chunk(0, c, fill=fill):
                        pass
                    att_chunks_left -= 1
                # no bulk drain between batches: att(1,0) is window-driven —
                # each key window only needs its own v chunk emitted, so the
                # leftover projection tops up per-window instead of flushing
                # serially while the PE idles.
                for c in range(4):
                    last = c == 3
                    drain_to(vpos[0] if c == 0 else max(vpos[3], qpos[c]))
                    fill = min(4, max(1, -(-len(fillers) // (16 * att_chunks_left))))
                    g = att_chunk(1, c, fill=fill, inline_out=last)
                    if c == 0:
                        next(g)  # window 0
                        for w in range(1, 4):
                            drain_to(vpos[w])
                            next(g, None)
                    for _ in g:
                        pass
                    att_chunks_left -= 1
                drain()


            if loop_reps > 1:
                with tc.For_i(0, loop_reps, 1):
                    emit_body()
            else:
                emit_body()

    nc.compile()
    return nc


_NC_CACHE = None


def _get_nc():
    global _NC_CACHE
    if _NC_CACHE is None:
        _NC_CACHE = _build()
    return _NC_CACHE


def _in_maps(x, w_in, b_in, w_out):
    x_flat = np.asarray(x, dtype=np.float32).reshape(NT, C)
    xT = np.ascontiguousarray(x_flat.T).astype(NP_BF16)
    w_in = np.asarray(w_in, dtype=np.float32)
    b_in = np.asarray(b_in, dtype=np.float32)
    w_out = np.asarray(w_out, dtype=np.float32)
    maps = []
    for c in range(NCORES):
        h0, h1 = 2 * c, 2 * c + 1
        rows = np.r_[h0 * 64:(h0 + 1) * 64, h1 * 64:(h1 + 1) * 64]
        wq = w_in[rows, :]
        wk = w_in[C + rows, :]
        wv = w_in[2 * C + rows, :]
        wqkvT = np.ascontiguousarray(
            np.concatenate([wq, wk, wv], 0).T
        ).astype(NP_BF16)
        bqkv = np.ascontiguousarray(
            np.stack([b_in[rows], b_in[C + rows], b_in[2 * C + rows]], 1)
        )
        bvrow = np.ascontiguousarray(b_in[2 * C + rows][None, :])
        woutT = np.ascontiguousarray(w_out[:, rows].T).astype(NP_BF16)
        maps.append({"xT": xT, "wqkvT": wqkvT, "bqkv": bqkv,
                     "bvrow": bvrow, "woutT": woutT})
    return maps


def run_spmd(x, w_in, b_in, w_out, **kwargs):
    nc = _get_nc()
    maps = _in_maps(x, w_in, b_in, w_out)
    return run_bass_kernel_spmd(nc, maps, core_ids=list(range(NCORES)), **kwargs)


def kernel(x, w_in, b_in, w_out, b_out):
    res = run_spmd(x, w_in, b_in, w_out)
    yT = np.zeros((C, NT), dtype=np.float64)
    for r in res.results:
        yT += r["yT"].astype(np.float64)
    y = yT.T + np.asarray(b_out, dtype=np.float64)[None, :]
    return y.reshape(B, N, C).astype(np.float32)
